# revision 12
# baseline (speedup 1.0000x reference)
"""Trainium2 Bass kernel for nn_GaussianBlurConv.

Model: sigma = MLP(x) per sample (Linear->ReLU->LN->Linear->ReLU->LN->Linear
->Sigmoid), then a per-sample normalized 5-tap gaussian depthwise conv along
the feature axis.

Strategy (8 NeuronCores, data-parallel over batch; 1024 rows/core):
  - mm1 (x @ W1, the dominant 17 GFLOP/core) runs on the tensor engine in
    float32r (full-rate fp32-storage matmul) with x^T pre-transposed on the
    host and resident in SBUF; W1 streamed from HBM exactly once.
  - b1 is added via a rank-1 matmul (ones ⊗ b1) into the same PSUM
    accumulation group.
  - relu + LN1 statistics are taken during PSUM evacuation on the scalar
    engine (accum_out), and LN1's affine normalization is *folded through*
    mm2: h2 = rs*(relu(h)@ (g1*W2)) - (m*rs)*colsum(g1*W2) + (be1@W2+b2).
    This makes relu(h) transient (PE-transposed 128x128 blocks feed mm2
    immediately) - no h residency, no second W1 pass.
  - LN2 is likewise folded through W3 into the sigmoid argument.
  - The 5-tap conv uses kernel symmetry: out = kc*(x + e1*(x<<1 + x>>1)
    + e4*(x<<2 + x>>2)) with e1=exp(-0.5/s^2), e4=exp(-2/s^2),
    kc=1/(1+2e1+2e4); 4 vector-engine ops + 1 scalar-engine prescale per
    chunk, with per-partition scalars (batch lives on partitions).
"""

import os
import sys

sys.path.insert(0, "/opt/trn_rl_repo")

from contextlib import ExitStack

import numpy as np

import concourse.bass as bass  # noqa: F401  (registers types)
import concourse.mybir as mybir
import concourse.tile as tile
from concourse import bacc
from concourse.bass_utils import run_bass_kernel_spmd
from concourse.masks import make_identity

F32 = mybir.dt.float32
F32R = mybir.dt.float32r
AF = mybir.ActivationFunctionType
ALU = mybir.AluOpType
AX = mybir.AxisListType

N_CORES = 8
LN_EPS = 1e-5


def build_program(BC, S, H1, C_val, D_val, conv_chunk=1024):
    """Emit the per-core SPMD program. BC = batch rows per core."""
    H2 = 64
    NB = BC // 128  # batch tiles
    ST = S // 128  # contraction (s) tiles
    SG = S // 512  # W1 load groups (4 s-tiles each)
    HC = H1 // 512  # h1 chunks of 512
    JB = H1 // 128  # h1 128-blocks total
    CQ = min(conv_chunk, S)  # conv chunk width
    NQ = S // CQ
    inv_h1 = 1.0 / H1
    inv_h2 = 1.0 / H2

    STAGE = int(os.environ.get("K_STAGE", "4"))
    TAIL = int(os.environ.get("K_TAIL", "99"))
    SALT = float(os.environ.get("K_SALT", "0"))
    nc = bacc.Bacc(None, target_bir_lowering=False)
    xt_d = nc.dram_tensor("xt", [S, BC], F32R, kind="ExternalInput")
    xp_d = nc.dram_tensor("xp", [BC, S + 4], F32, kind="ExternalInput")
    w1_d = nc.dram_tensor("w1", [S, H1], F32R, kind="ExternalInput")
    b1_d = nc.dram_tensor("b1b", [128, H1], F32, kind="ExternalInput")
    w2g_d = nc.dram_tensor("w2g", [H1, H2], F32, kind="ExternalInput")
    gw2c_d = nc.dram_tensor("gw2c", [128, H2], F32, kind="ExternalInput")
    cb2_d = nc.dram_tensor("cb2", [128, H2], F32, kind="ExternalInput")
    w3g_d = nc.dram_tensor("w3g", [128, H2], F32, kind="ExternalInput")
    out_d = nc.dram_tensor("out", [BC, S], F32, kind="ExternalOutput")

    with tile.TileContext(nc) as tc, ExitStack() as ctx:
        cpool = ctx.enter_context(tc.tile_pool(name="consts", bufs=1))
        xtp = ctx.enter_context(tc.tile_pool(name="xt", bufs=1))
        w1p = ctx.enter_context(tc.tile_pool(name="w1", bufs=2))
        hrp = ctx.enter_context(tc.tile_pool(name="hr", bufs=2))
        hrtp = ctx.enter_context(tc.tile_pool(name="hrt", bufs=2))
        sqp = ctx.enter_context(tc.tile_pool(name="sq", bufs=1))
        stp = ctx.enter_context(tc.tile_pool(name="stats", bufs=1))
        smp = ctx.enter_context(tc.tile_pool(name="smalls", bufs=16))
        b1p = ctx.enter_context(tc.tile_pool(name="b1p", bufs=2))
        sm64 = ctx.enter_context(tc.tile_pool(name="sm64", bufs=4))
        krp = ctx.enter_context(tc.tile_pool(name="kern", bufs=2))
        cxp = ctx.enter_context(tc.tile_pool(name="convx", bufs=2))
        ctp = ctx.enter_context(tc.tile_pool(name="convt", bufs=1))
        psp = ctx.enter_context(tc.tile_pool(name="ps", bufs=8, space="PSUM"))

        # ---- constants ----
        ident = cpool.tile([128, 128], F32)
        make_identity(nc, ident[:])
        w2g = cpool.tile([128, JB, H2], F32)
        nc.sync.dma_start(
            w2g[:], w2g_d.ap().rearrange("(j p) n -> p j n", p=128)
        )
        gw2c = cpool.tile([128, H2], F32)
        nc.sync.dma_start(gw2c[:], gw2c_d[:])
        cb2 = cpool.tile([128, H2], F32)
        nc.sync.dma_start(cb2[:], cb2_d[:])
        w3g = cpool.tile([128, H2], F32)
        nc.sync.dma_start(w3g[:], w3g_d[:])
        eps_t = cpool.tile([128, 1], F32)
        nc.vector.memset(eps_t[:], LN_EPS)
        salt_t = cpool.tile([128, 1], F32)
        nc.vector.memset(salt_t[:], 1.0 + SALT)
        dv_t = cpool.tile([128, 1], F32)
        nc.vector.memset(dv_t[:], D_val)

        # ---- resident x^T (one tile per 128-row s-block) ----
        xt_tiles = []
        for so in range(ST):
            t = xtp.tile([128, BC], F32R, tag=f"xt{so}")
            nc.sync.dma_start(t[:], xt_d[so * 128 : (so + 1) * 128, :])
            xt_tiles.append(t)

        # ---- persistent per-batch-tile stats / h2 accumulators ----
        sums = stp.tile([128, NB * HC], F32)
        sqs = stp.tile([128, NB * HC], F32)
        h2sb = stp.tile([128, NB * H2], F32)

        # ---- main pipeline over h1 chunks ----
        for c in range(HC):
            ps_mm1 = [psp.tile([128, 512], F32, tag="ps", name=f"psmm1_{c}_{i}") for i in range(NB)]
            for sg in range(SG):
                w1g = w1p.tile([128, 4, 512], F32R)
                nc.sync.dma_start(
                    w1g[:],
                    w1_d[sg * 512 : (sg + 1) * 512, c * 512 : (c + 1) * 512]
                    .rearrange("(t p) n -> p t n", p=128),
                )
                for t in range(4):
                    so = sg * 4 + t
                    for b in range(NB):
                        nc.tensor.matmul(
                            ps_mm1[b][:],
                            xt_tiles[so][:, b * 128 : (b + 1) * 128],
                            w1g[:, t : t + 1, :],
                            start=(so == 0),
                            stop=(so == ST - 1),
                        )
            b1c = b1p.tile([128, 512], F32, tag="b1c")
            nc.sync.dma_start(b1c[:], b1_d[:, c * 512 : (c + 1) * 512])
            for b in range(NB):
                nc.vector.tensor_tensor(
                    ps_mm1[b][:], ps_mm1[b][:], b1c[:], ALU.add
                )
            for b in range(NB):
                k = b * HC + c
                hr = hrp.tile([128, 512], F32)
                nc.scalar.activation(
                    hr[:], ps_mm1[b][:], AF.Relu,
                    accum_out=sums[:, k : k + 1],
                )
                scr = sqp.tile([128, 512], F32)
                nc.scalar.activation(
                    scr[:], hr[:], AF.Square,
                    accum_out=sqs[:, k : k + 1],
                )
                if STAGE < 2:
                    continue
                pst = psp.tile([128, 512], F32, tag="ps")
                for j in range(4):
                    nc.tensor.transpose(
                        pst[:, j * 128 : (j + 1) * 128],
                        hr[:, j * 128 : (j + 1) * 128],
                        ident[:],
                    )
                hrt = hrtp.tile([128, 512], F32)
                nc.vector.tensor_copy(hrt[:], pst[:])
                ph2 = psp.tile([128, H2], F32, tag="ps")
                for j in range(4):
                    jj = c * 4 + j
                    nc.tensor.matmul(
                        ph2[:],
                        hrt[:, j * 128 : (j + 1) * 128],
                        w2g[:, jj : jj + 1, :],
                        start=(j == 0),
                        stop=(j == 3),
                    )
                dst = h2sb[:, b * H2 : (b + 1) * H2]
                if c == 0:
                    nc.vector.tensor_copy(dst, ph2[:])
                else:
                    nc.vector.tensor_tensor(dst, dst, ph2[:], ALU.add)

        # ---- per-batch-tile tail: LN folds, sigma, kernel, conv ----
        for b in range(NB):
            if STAGE < 3:
                break
            # LN1 stats
            ssum = smp.tile([128, 1], F32, tag="sm")
            nc.vector.tensor_reduce(
                ssum[:], sums[:, b * HC : (b + 1) * HC], axis=AX.X, op=ALU.add
            )
            m = smp.tile([128, 1], F32, tag="sm")
            nc.vector.tensor_scalar_mul(m[:], ssum[:], inv_h1)
            qsum = smp.tile([128, 1], F32, tag="sm")
            nc.vector.tensor_reduce(
                qsum[:], sqs[:, b * HC : (b + 1) * HC], axis=AX.X, op=ALU.add
            )
            msq = smp.tile([128, 1], F32, tag="sm")
            nc.vector.tensor_tensor(msq[:], m[:], m[:], ALU.mult)
            var = smp.tile([128, 1], F32, tag="sm")
            nc.vector.scalar_tensor_tensor(
                var[:], qsum[:], inv_h1, msq[:], ALU.mult, ALU.subtract
            )
            sd = smp.tile([128, 1], F32, tag="sm")
            nc.scalar.activation(sd[:], var[:], AF.Sqrt, bias=eps_t[:])
            rs = smp.tile([128, 1], F32, tag="sm")
            nc.vector.reciprocal(rs[:], sd[:])
            nrs = smp.tile([128, 1], F32, tag="sm")
            nc.vector.tensor_scalar_mul(nrs[:], rs[:], -1.0)
            mnrs = smp.tile([128, 1], F32, tag="sm")
            nc.vector.tensor_tensor(mnrs[:], m[:], nrs[:], ALU.mult)
            if TAIL < 2:
                continue
            # fold LN1 into mm2 output
            u64 = sm64.tile([128, H2], F32, tag="s64")
            nc.vector.tensor_scalar_mul(
                u64[:], h2sb[:, b * H2 : (b + 1) * H2], rs[:]
            )
            w64 = sm64.tile([128, H2], F32, tag="s64")
            nc.vector.scalar_tensor_tensor(
                w64[:], gw2c[:], mnrs[:], u64[:], ALU.mult, ALU.add
            )
            h2p = sm64.tile([128, H2], F32, tag="s64")
            nc.vector.tensor_tensor(h2p[:], w64[:], cb2[:], ALU.add)
            if TAIL < 3:
                continue
            # relu + LN2 stats
            h2r = sm64.tile([128, H2], F32, tag="s64")
            s2t = smp.tile([128, 1], F32, tag="sm")
            nc.scalar.activation(
                h2r[:], h2p[:], AF.Relu, accum_out=s2t[:]
            )
            scr2 = sm64.tile([128, H2], F32, tag="s64")
            q2t = smp.tile([128, 1], F32, tag="sm")
            nc.scalar.activation(
                scr2[:], h2r[:], AF.Square, accum_out=q2t[:]
            )
            m2 = smp.tile([128, 1], F32, tag="sm")
            nc.vector.tensor_scalar_mul(m2[:], s2t[:], inv_h2)
            msq2 = smp.tile([128, 1], F32, tag="sm")
            nc.vector.tensor_tensor(msq2[:], m2[:], m2[:], ALU.mult)
            var2 = smp.tile([128, 1], F32, tag="sm")
            nc.vector.scalar_tensor_tensor(
                var2[:], q2t[:], inv_h2, msq2[:], ALU.mult, ALU.subtract
            )
            sd2 = smp.tile([128, 1], F32, tag="sm")
            nc.scalar.activation(sd2[:], var2[:], AF.Sqrt, bias=eps_t[:])
            rs2 = smp.tile([128, 1], F32, tag="sm")
            nc.vector.reciprocal(rs2[:], sd2[:])
            if TAIL < 4:
                continue
            # mm3 (LN2 folded): logit = rs2*lraw - m2*rs2*C + D
            scr3 = sm64.tile([128, H2], F32, tag="s64")
            nc.vector.tensor_tensor(scr3[:], h2r[:], w3g[:], ALU.mult)
            lraw = smp.tile([128, 1], F32, tag="sm")
            nc.vector.tensor_reduce(lraw[:], scr3[:], axis=AX.X, op=ALU.add)
            s1 = smp.tile([128, 1], F32, tag="sm")
            nc.vector.tensor_tensor(s1[:], lraw[:], rs2[:], ALU.mult)
            s2m = smp.tile([128, 1], F32, tag="sm")
            nc.vector.tensor_tensor(s2m[:], m2[:], rs2[:], ALU.mult)
            lp = smp.tile([128, 1], F32, tag="sm")
            nc.vector.scalar_tensor_tensor(
                lp[:], s2m[:], -C_val, s1[:], ALU.mult, ALU.add
            )
            sig = smp.tile([128, 1], F32, tag="sm")
            nc.scalar.activation(sig[:], lp[:], AF.Sigmoid, bias=dv_t[:])
            if TAIL < 5:
                continue
            # gaussian taps: e1 = exp(-0.5/s^2), e4 = exp(-2/s^2)
            s2sig = smp.tile([128, 1], F32, tag="sm")
            nc.vector.tensor_tensor(s2sig[:], sig[:], sig[:], ALU.mult)
            rinv = smp.tile([128, 1], F32, tag="sm")
            nc.vector.reciprocal(rinv[:], s2sig[:])
            e1 = krp.tile([128, 1], F32, tag="e1")
            nc.scalar.activation(e1[:], rinv[:], AF.Exp, scale=-0.5)
            e4 = krp.tile([128, 1], F32, tag="e4")
            nc.scalar.activation(e4[:], rinv[:], AF.Exp, scale=-2.0)
            esum = smp.tile([128, 1], F32, tag="sm")
            nc.vector.tensor_tensor(esum[:], e1[:], e4[:], ALU.add)
            den = smp.tile([128, 1], F32, tag="sm")
            nc.vector.tensor_scalar(den[:], esum[:], 2.0, 1.0, ALU.mult, ALU.add)
            kc = krp.tile([128, 1], F32, tag="kc")
            nc.vector.reciprocal(kc[:], den[:])
            # conv: out = kc*x + e1*(kc*x)<<>>1 + e4*(kc*x)<<>>2
            for q in range(NQ if STAGE >= 4 else 0):
                xs = cxp.tile([128, CQ + 4], F32, tag="xs")
                nc.sync.dma_start(
                    xs[:],
                    xp_d[b * 128 : (b + 1) * 128, q * CQ : q * CQ + CQ + 4],
                )
                nc.scalar.activation(xs[:], xs[:], AF.Copy, scale=kc[:])
                t1 = ctp.tile([128, CQ], F32, tag="t1")
                nc.vector.tensor_tensor(
                    t1[:], xs[:, 1 : CQ + 1], xs[:, 3 : CQ + 3], ALU.add
                )
                t2 = ctp.tile([128, CQ], F32, tag="t2")
                nc.vector.tensor_tensor(
                    t2[:], xs[:, 0:CQ], xs[:, 4 : CQ + 4], ALU.add
                )
                nc.vector.scalar_tensor_tensor(
                    t1[:], t1[:], e1[:], xs[:, 2 : CQ + 2], ALU.mult, ALU.add
                )
                nc.vector.scalar_tensor_tensor(
                    xs[:, 2 : CQ + 2], t2[:], e4[:], t1[:], ALU.mult, ALU.add
                )
                nc.sync.dma_start(
                    out_d[b * 128 : (b + 1) * 128, q * CQ : (q + 1) * CQ],
                    xs[:, 2 : CQ + 2],
                )

    nc.compile()
    return nc


def prep_inputs(x, W1, b1, g1, be1, W2, b2, g2, be2, W3, b3, n_cores=N_CORES):
    """Host-side sharding + derived-constant computation."""
    B, S = x.shape
    H1 = W1.shape[1]
    BC = B // n_cores
    f = np.float32

    W2g = (W2 * g1[:, None]).astype(f)  # [H1, H2]
    gw2c = np.tile((g1 @ W2).astype(f)[None, :], (128, 1)).astype(f)
    cb2 = np.tile((be1 @ W2 + b2).astype(f)[None, :], (128, 1)).astype(f)
    w3g = np.tile((g2 * W3[:, 0]).astype(f)[None, :], (128, 1)).astype(f)
    C_val = float(np.sum(g2 * W3[:, 0], dtype=np.float64))
    D_val = float(np.dot(be2, W3[:, 0]) + b3[0])

    in_maps = []
    for c in range(n_cores):
        xs = x[c * BC : (c + 1) * BC]
        xt = np.ascontiguousarray(xs.T)
        xp = np.zeros((BC, S + 4), f)
        xp[:, 2 : S + 2] = xs
        in_maps.append(
            {
                "xt": xt,
                "xp": xp,
                "w1": np.ascontiguousarray(W1.astype(f)),
                "b1b": np.ascontiguousarray(
                    np.broadcast_to(b1.astype(f)[None, :], (128, len(b1)))
                ),
                "w2g": W2g,
                "gw2c": gw2c,
                "cb2": cb2,
                "w3g": w3g,
            }
        )
    return in_maps, C_val, D_val, BC, S, H1


def kernel(x, W1, b1, g1, be1, W2, b2, g2, be2, W3, b3):
    in_maps, C_val, D_val, BC, S, H1 = prep_inputs(
        x, W1, b1, g1, be1, W2, b2, g2, be2, W3, b3
    )
    nc = build_program(BC, S, H1, C_val, D_val)
    res = run_bass_kernel_spmd(nc, in_maps, core_ids=list(range(N_CORES)))
    return np.concatenate([r["out"] for r in res.results], axis=0)


# revision 13
# speedup vs baseline: 15.2833x; 15.2833x over previous
"""Trainium2 Bass kernel for nn_GaussianBlurConv.

Model: sigma = MLP(x) per sample (Linear->ReLU->LN->Linear->ReLU->LN->Linear
->Sigmoid), then a per-sample normalized 5-tap gaussian depthwise conv along
the feature axis.

Strategy (8 NeuronCores, data-parallel over batch; 1024 rows/core):
  - mm1 (x @ W1, the dominant 17 GFLOP/core) runs on the tensor engine in
    float32r (full-rate fp32-storage matmul) with x^T pre-transposed on the
    host and resident in SBUF; W1 streamed from HBM exactly once.
  - b1 is added via a rank-1 matmul (ones ⊗ b1) into the same PSUM
    accumulation group.
  - relu + LN1 statistics are taken during PSUM evacuation on the scalar
    engine (accum_out), and LN1's affine normalization is *folded through*
    mm2: h2 = rs*(relu(h)@ (g1*W2)) - (m*rs)*colsum(g1*W2) + (be1@W2+b2).
    This makes relu(h) transient (PE-transposed 128x128 blocks feed mm2
    immediately) - no h residency, no second W1 pass.
  - LN2 is likewise folded through W3 into the sigmoid argument.
  - The 5-tap conv uses kernel symmetry: out = kc*(x + e1*(x<<1 + x>>1)
    + e4*(x<<2 + x>>2)) with e1=exp(-0.5/s^2), e4=exp(-2/s^2),
    kc=1/(1+2e1+2e4); 4 vector-engine ops + 1 scalar-engine prescale per
    chunk, with per-partition scalars (batch lives on partitions).
"""

import os
import sys

sys.path.insert(0, "/opt/trn_rl_repo")

from contextlib import ExitStack

import numpy as np

import concourse.bass as bass  # noqa: F401  (registers types)
import concourse.mybir as mybir
import concourse.tile as tile
from concourse import bacc
from concourse.bass_utils import run_bass_kernel_spmd
from concourse.masks import make_identity

F32 = mybir.dt.float32
F32R = mybir.dt.float32r
AF = mybir.ActivationFunctionType
ALU = mybir.AluOpType
AX = mybir.AxisListType

N_CORES = 8
LN_EPS = 1e-5


def build_program(BC, S, H1, C_val, D_val, conv_chunk=1024):
    """Emit the per-core SPMD program. BC = batch rows per core."""
    H2 = 64
    NB = BC // 128  # batch tiles
    ST = S // 128  # contraction (s) tiles
    SG = S // 512  # W1 load groups (4 s-tiles each)
    HC = H1 // 512  # h1 chunks of 512
    JB = H1 // 128  # h1 128-blocks total
    CQ = min(conv_chunk, S)  # conv chunk width
    NQ = S // CQ
    inv_h1 = 1.0 / H1
    inv_h2 = 1.0 / H2

    STAGE = int(os.environ.get("K_STAGE", "4"))
    TAIL = int(os.environ.get("K_TAIL", "99"))
    SALT = float(os.environ.get("K_SALT", "0"))
    nc = bacc.Bacc(None, target_bir_lowering=False)
    xt_d = nc.dram_tensor("xt", [S, BC], F32R, kind="ExternalInput")
    xp_d = nc.dram_tensor("xp", [BC, S + 4], F32, kind="ExternalInput")
    w1_d = nc.dram_tensor("w1", [S, H1], F32R, kind="ExternalInput")
    b1_d = nc.dram_tensor("b1b", [128, H1], F32, kind="ExternalInput")
    w2g_d = nc.dram_tensor("w2g", [H1, H2], F32, kind="ExternalInput")
    gw2c_d = nc.dram_tensor("gw2c", [128, H2], F32, kind="ExternalInput")
    cb2_d = nc.dram_tensor("cb2", [128, H2], F32, kind="ExternalInput")
    w3g_d = nc.dram_tensor("w3g", [128, H2], F32, kind="ExternalInput")
    out_d = nc.dram_tensor("out", [BC, S], F32, kind="ExternalOutput")

    with tile.TileContext(nc) as tc, ExitStack() as ctx:
        cpool = ctx.enter_context(tc.tile_pool(name="consts", bufs=1))
        xtp = ctx.enter_context(tc.tile_pool(name="xt", bufs=1))
        w1p = ctx.enter_context(tc.tile_pool(name="w1", bufs=2))
        hrp = ctx.enter_context(tc.tile_pool(name="hr", bufs=2))
        hrtp = ctx.enter_context(tc.tile_pool(name="hrt", bufs=2))
        sqp = ctx.enter_context(tc.tile_pool(name="sq", bufs=1))
        stp = ctx.enter_context(tc.tile_pool(name="stats", bufs=1))
        smp = ctx.enter_context(tc.tile_pool(name="smalls", bufs=16))
        b1p = ctx.enter_context(tc.tile_pool(name="b1p", bufs=2))
        sm64 = ctx.enter_context(tc.tile_pool(name="sm64", bufs=4))
        krp = ctx.enter_context(tc.tile_pool(name="kern", bufs=2))
        cxp = ctx.enter_context(tc.tile_pool(name="convx", bufs=2))
        ctp = ctx.enter_context(tc.tile_pool(name="convt", bufs=1))
        psp = ctx.enter_context(tc.tile_pool(name="ps", bufs=8, space="PSUM"))

        # ---- constants ----
        ident = cpool.tile([128, 128], F32)
        make_identity(nc, ident[:])
        w2g = cpool.tile([128, JB, H2], F32)
        nc.sync.dma_start(
            w2g[:], w2g_d.ap().rearrange("(j p) n -> p j n", p=128)
        )
        gw2c = cpool.tile([128, H2], F32)
        nc.sync.dma_start(gw2c[:], gw2c_d[:])
        cb2 = cpool.tile([128, H2], F32)
        nc.sync.dma_start(cb2[:], cb2_d[:])
        w3g = cpool.tile([128, H2], F32)
        nc.sync.dma_start(w3g[:], w3g_d[:])
        eps_t = cpool.tile([128, 1], F32)
        nc.vector.memset(eps_t[:], LN_EPS)
        salt_t = cpool.tile([128, 1], F32)
        nc.vector.memset(salt_t[:], 1.0 + SALT)
        dv_t = cpool.tile([128, 1], F32)
        nc.vector.memset(dv_t[:], D_val)

        # ---- resident x^T (one tile per 128-row s-block) ----
        xt_tiles = []
        for so in range(ST):
            t = xtp.tile([128, BC], F32R, tag=f"xt{so}")
            nc.sync.dma_start(t[:], xt_d[so * 128 : (so + 1) * 128, :])
            xt_tiles.append(t)

        # ---- persistent per-batch-tile stats / h2 accumulators ----
        sums = stp.tile([128, NB * HC], F32)
        sqs = stp.tile([128, NB * HC], F32)
        h2sb = stp.tile([128, NB * H2], F32)

        # ---- main pipeline over h1 chunks ----
        for c in range(HC):
            ps_mm1 = [psp.tile([128, 512], F32, tag="ps", name=f"psmm1_{c}_{i}") for i in range(NB)]
            for sg in range(SG):
                w1g = w1p.tile([128, 4, 512], F32R)
                nc.sync.dma_start(
                    w1g[:],
                    w1_d[sg * 512 : (sg + 1) * 512, c * 512 : (c + 1) * 512]
                    .rearrange("(t p) n -> p t n", p=128),
                )
                for t in range(4):
                    so = sg * 4 + t
                    for b in range(NB):
                        nc.tensor.matmul(
                            ps_mm1[b][:],
                            xt_tiles[so][:, b * 128 : (b + 1) * 128],
                            w1g[:, t : t + 1, :],
                            start=(so == 0),
                            stop=(so == ST - 1),
                        )
            b1c = b1p.tile([128, 512], F32, tag="b1c")
            nc.sync.dma_start(b1c[:], b1_d[:, c * 512 : (c + 1) * 512])
            for b in range(NB):
                k = b * HC + c
                hpre = hrp.tile([128, 512], F32, tag="hpre")
                nc.vector.tensor_tensor(
                    hpre[:], ps_mm1[b][:], b1c[:], ALU.add
                )
                hr = hrp.tile([128, 512], F32, tag="hr")
                nc.scalar.activation(
                    hr[:], hpre[:], AF.Relu,
                    accum_out=sums[:, k : k + 1],
                )
                nc.scalar.activation(
                    hpre[:], hr[:], AF.Square,
                    accum_out=sqs[:, k : k + 1],
                )
                if STAGE < 2:
                    continue
                for j in range(4):
                    nc.tensor.transpose(
                        ps_mm1[b][:, j * 128 : (j + 1) * 128],
                        hr[:, j * 128 : (j + 1) * 128],
                        ident[:],
                    )
                hrt = hrtp.tile([128, 512], F32)
                nc.vector.tensor_copy(hrt[:], ps_mm1[b][:])
                for j in range(4):
                    jj = c * 4 + j
                    nc.tensor.matmul(
                        ps_mm1[b][:, 0:H2],
                        hrt[:, j * 128 : (j + 1) * 128],
                        w2g[:, jj : jj + 1, :],
                        start=(j == 0),
                        stop=(j == 3),
                    )
                dst = h2sb[:, b * H2 : (b + 1) * H2]
                if c == 0:
                    nc.vector.tensor_copy(dst, ps_mm1[b][:, 0:H2])
                else:
                    nc.vector.tensor_tensor(dst, dst, ps_mm1[b][:, 0:H2], ALU.add)

        # ---- per-batch-tile tail: LN folds, sigma, kernel, conv ----
        for b in range(NB):
            if STAGE < 3:
                break
            # LN1 stats
            ssum = smp.tile([128, 1], F32, tag="sm")
            nc.vector.tensor_reduce(
                ssum[:], sums[:, b * HC : (b + 1) * HC], axis=AX.X, op=ALU.add
            )
            m = smp.tile([128, 1], F32, tag="sm")
            nc.vector.tensor_scalar_mul(m[:], ssum[:], inv_h1)
            qsum = smp.tile([128, 1], F32, tag="sm")
            nc.vector.tensor_reduce(
                qsum[:], sqs[:, b * HC : (b + 1) * HC], axis=AX.X, op=ALU.add
            )
            msq = smp.tile([128, 1], F32, tag="sm")
            nc.vector.tensor_tensor(msq[:], m[:], m[:], ALU.mult)
            var = smp.tile([128, 1], F32, tag="sm")
            nc.vector.scalar_tensor_tensor(
                var[:], qsum[:], inv_h1, msq[:], ALU.mult, ALU.subtract
            )
            sd = smp.tile([128, 1], F32, tag="sm")
            nc.scalar.activation(sd[:], var[:], AF.Sqrt, bias=eps_t[:])
            rs = smp.tile([128, 1], F32, tag="sm")
            nc.vector.reciprocal(rs[:], sd[:])
            nrs = smp.tile([128, 1], F32, tag="sm")
            nc.vector.tensor_scalar_mul(nrs[:], rs[:], -1.0)
            mnrs = smp.tile([128, 1], F32, tag="sm")
            nc.vector.tensor_tensor(mnrs[:], m[:], nrs[:], ALU.mult)
            if TAIL < 2:
                continue
            # fold LN1 into mm2 output
            u64 = sm64.tile([128, H2], F32, tag="s64")
            nc.vector.tensor_scalar_mul(
                u64[:], h2sb[:, b * H2 : (b + 1) * H2], rs[:]
            )
            w64 = sm64.tile([128, H2], F32, tag="s64")
            nc.vector.scalar_tensor_tensor(
                w64[:], gw2c[:], mnrs[:], u64[:], ALU.mult, ALU.add
            )
            h2p = sm64.tile([128, H2], F32, tag="s64")
            nc.vector.tensor_tensor(h2p[:], w64[:], cb2[:], ALU.add)
            if TAIL < 3:
                continue
            # relu + LN2 stats
            h2r = sm64.tile([128, H2], F32, tag="s64")
            s2t = smp.tile([128, 1], F32, tag="sm")
            nc.scalar.activation(
                h2r[:], h2p[:], AF.Relu, accum_out=s2t[:]
            )
            scr2 = sm64.tile([128, H2], F32, tag="s64")
            q2t = smp.tile([128, 1], F32, tag="sm")
            nc.scalar.activation(
                scr2[:], h2r[:], AF.Square, accum_out=q2t[:]
            )
            m2 = smp.tile([128, 1], F32, tag="sm")
            nc.vector.tensor_scalar_mul(m2[:], s2t[:], inv_h2)
            msq2 = smp.tile([128, 1], F32, tag="sm")
            nc.vector.tensor_tensor(msq2[:], m2[:], m2[:], ALU.mult)
            var2 = smp.tile([128, 1], F32, tag="sm")
            nc.vector.scalar_tensor_tensor(
                var2[:], q2t[:], inv_h2, msq2[:], ALU.mult, ALU.subtract
            )
            sd2 = smp.tile([128, 1], F32, tag="sm")
            nc.scalar.activation(sd2[:], var2[:], AF.Sqrt, bias=eps_t[:])
            rs2 = smp.tile([128, 1], F32, tag="sm")
            nc.vector.reciprocal(rs2[:], sd2[:])
            if TAIL < 4:
                continue
            # mm3 (LN2 folded): logit = rs2*lraw - m2*rs2*C + D
            scr3 = sm64.tile([128, H2], F32, tag="s64")
            nc.vector.tensor_tensor(scr3[:], h2r[:], w3g[:], ALU.mult)
            lraw = smp.tile([128, 1], F32, tag="sm")
            nc.vector.tensor_reduce(lraw[:], scr3[:], axis=AX.X, op=ALU.add)
            s1 = smp.tile([128, 1], F32, tag="sm")
            nc.vector.tensor_tensor(s1[:], lraw[:], rs2[:], ALU.mult)
            s2m = smp.tile([128, 1], F32, tag="sm")
            nc.vector.tensor_tensor(s2m[:], m2[:], rs2[:], ALU.mult)
            lp = smp.tile([128, 1], F32, tag="sm")
            nc.vector.scalar_tensor_tensor(
                lp[:], s2m[:], -C_val, s1[:], ALU.mult, ALU.add
            )
            sig = smp.tile([128, 1], F32, tag="sm")
            nc.scalar.activation(sig[:], lp[:], AF.Sigmoid, bias=dv_t[:])
            if TAIL < 5:
                continue
            # gaussian taps: e1 = exp(-0.5/s^2), e4 = exp(-2/s^2)
            s2sig = smp.tile([128, 1], F32, tag="sm")
            nc.vector.tensor_tensor(s2sig[:], sig[:], sig[:], ALU.mult)
            rinv = smp.tile([128, 1], F32, tag="sm")
            nc.vector.reciprocal(rinv[:], s2sig[:])
            e1 = krp.tile([128, 1], F32, tag="e1")
            nc.scalar.activation(e1[:], rinv[:], AF.Exp, scale=-0.5)
            e4 = krp.tile([128, 1], F32, tag="e4")
            nc.scalar.activation(e4[:], rinv[:], AF.Exp, scale=-2.0)
            esum = smp.tile([128, 1], F32, tag="sm")
            nc.vector.tensor_tensor(esum[:], e1[:], e4[:], ALU.add)
            den = smp.tile([128, 1], F32, tag="sm")
            nc.vector.tensor_scalar(den[:], esum[:], 2.0, 1.0, ALU.mult, ALU.add)
            kc = krp.tile([128, 1], F32, tag="kc")
            nc.vector.reciprocal(kc[:], den[:])
            # conv: out = kc*x + e1*(kc*x)<<>>1 + e4*(kc*x)<<>>2
            for q in range(NQ if STAGE >= 4 else 0):
                xs = cxp.tile([128, CQ + 4], F32, tag="xs")
                nc.sync.dma_start(
                    xs[:],
                    xp_d[b * 128 : (b + 1) * 128, q * CQ : q * CQ + CQ + 4],
                )
                nc.scalar.activation(xs[:], xs[:], AF.Copy, scale=kc[:])
                t1 = ctp.tile([128, CQ], F32, tag="t1")
                nc.vector.tensor_tensor(
                    t1[:], xs[:, 1 : CQ + 1], xs[:, 3 : CQ + 3], ALU.add
                )
                t2 = ctp.tile([128, CQ], F32, tag="t2")
                nc.vector.tensor_tensor(
                    t2[:], xs[:, 0:CQ], xs[:, 4 : CQ + 4], ALU.add
                )
                nc.vector.scalar_tensor_tensor(
                    t1[:], t1[:], e1[:], xs[:, 2 : CQ + 2], ALU.mult, ALU.add
                )
                nc.vector.scalar_tensor_tensor(
                    xs[:, 2 : CQ + 2], t2[:], e4[:], t1[:], ALU.mult, ALU.add
                )
                nc.sync.dma_start(
                    out_d[b * 128 : (b + 1) * 128, q * CQ : (q + 1) * CQ],
                    xs[:, 2 : CQ + 2],
                )

    nc.compile()
    return nc


def prep_inputs(x, W1, b1, g1, be1, W2, b2, g2, be2, W3, b3, n_cores=N_CORES):
    """Host-side sharding + derived-constant computation."""
    B, S = x.shape
    H1 = W1.shape[1]
    BC = B // n_cores
    f = np.float32

    W2g = (W2 * g1[:, None]).astype(f)  # [H1, H2]
    gw2c = np.tile((g1 @ W2).astype(f)[None, :], (128, 1)).astype(f)
    cb2 = np.tile((be1 @ W2 + b2).astype(f)[None, :], (128, 1)).astype(f)
    w3g = np.tile((g2 * W3[:, 0]).astype(f)[None, :], (128, 1)).astype(f)
    C_val = float(np.sum(g2 * W3[:, 0], dtype=np.float64))
    D_val = float(np.dot(be2, W3[:, 0]) + b3[0])

    in_maps = []
    for c in range(n_cores):
        xs = x[c * BC : (c + 1) * BC]
        xt = np.ascontiguousarray(xs.T)
        xp = np.zeros((BC, S + 4), f)
        xp[:, 2 : S + 2] = xs
        in_maps.append(
            {
                "xt": xt,
                "xp": xp,
                "w1": np.ascontiguousarray(W1.astype(f)),
                "b1b": np.ascontiguousarray(
                    np.broadcast_to(b1.astype(f)[None, :], (128, len(b1)))
                ),
                "w2g": W2g,
                "gw2c": gw2c,
                "cb2": cb2,
                "w3g": w3g,
            }
        )
    return in_maps, C_val, D_val, BC, S, H1


def kernel(x, W1, b1, g1, be1, W2, b2, g2, be2, W3, b3):
    in_maps, C_val, D_val, BC, S, H1 = prep_inputs(
        x, W1, b1, g1, be1, W2, b2, g2, be2, W3, b3
    )
    nc = build_program(BC, S, H1, C_val, D_val)
    res = run_bass_kernel_spmd(nc, in_maps, core_ids=list(range(N_CORES)))
    return np.concatenate([r["out"] for r in res.results], axis=0)


# revision 16
# speedup vs baseline: 27.2691x; 1.7842x over previous
"""Trainium2 Bass kernel for nn_GaussianBlurConv.

Model: sigma = MLP(x) per sample (Linear->ReLU->LN->Linear->ReLU->LN->Linear
->Sigmoid), then a per-sample normalized 5-tap gaussian depthwise conv along
the feature axis.

Strategy (8 NeuronCores, data-parallel over batch; 1024 rows/core):
  - mm1 (x @ W1, the dominant 17 GFLOP/core) runs on the tensor engine in
    float32r (full-rate fp32-storage matmul) with x^T pre-transposed on the
    host and resident in SBUF; W1 streamed from HBM exactly once.
  - b1 is added during PSUM evacuation on the vector engine (broadcast
    tile add), ahead of the relu on the scalar engine.
  - relu + LN1 statistics are taken during PSUM evacuation on the scalar
    engine (accum_out), and LN1's affine normalization is *folded through*
    mm2: h2 = rs*(relu(h)@ (g1*W2)) - (m*rs)*colsum(g1*W2) + (be1@W2+b2).
    This makes relu(h) transient (PE-transposed 128x128 blocks feed mm2
    immediately) - no h residency, no second W1 pass.
  - LN2 is likewise folded through W3 into the sigmoid argument.
  - The 5-tap conv uses kernel symmetry: out = kc*(x + e1*(x<<1 + x>>1)
    + e4*(x<<2 + x>>2)) with e1=exp(-0.5/s^2), e4=exp(-2/s^2),
    kc=1/(1+2e1+2e4); 4 vector-engine ops + 1 scalar-engine prescale per
    chunk, with per-partition scalars (batch lives on partitions).
"""

import os
import sys

sys.path.insert(0, "/opt/trn_rl_repo")

from contextlib import ExitStack

import numpy as np

import concourse.bass as bass  # noqa: F401  (registers types)
import concourse.mybir as mybir
import concourse.tile as tile
from concourse import bacc
from concourse.bass_utils import run_bass_kernel_spmd
from concourse.masks import make_identity

F32 = mybir.dt.float32
F32R = mybir.dt.float32r
AF = mybir.ActivationFunctionType
ALU = mybir.AluOpType
AX = mybir.AxisListType

N_CORES = 8
LN_EPS = 1e-5


def build_program(BC, S, H1, C_val, D_val, conv_chunk=1024):
    """Emit the per-core SPMD program. BC = batch rows per core."""
    H2 = 64
    NB = BC // 128  # batch tiles
    ST = S // 128  # contraction (s) tiles
    SG = S // 512  # W1 load groups (4 s-tiles each)
    HC = H1 // 512  # h1 chunks of 512
    JB = H1 // 128  # h1 128-blocks total
    CQ = min(conv_chunk, S)  # conv chunk width
    NQ = S // CQ
    inv_h1 = 1.0 / H1
    inv_h2 = 1.0 / H2

    STAGE = int(os.environ.get("K_STAGE", "4"))
    TAIL = int(os.environ.get("K_TAIL", "99"))
    SALT = float(os.environ.get("K_SALT", "0"))
    nc = bacc.Bacc(None, target_bir_lowering=False)
    xt_d = nc.dram_tensor("xt", [S, BC], F32R, kind="ExternalInput")
    xp_d = nc.dram_tensor("xp", [BC, S + 4], F32, kind="ExternalInput")
    w1_d = nc.dram_tensor("w1", [S, H1], F32R, kind="ExternalInput")
    b1_d = nc.dram_tensor("b1b", [128, H1], F32, kind="ExternalInput")
    w2g_d = nc.dram_tensor("w2g", [H1, H2], F32, kind="ExternalInput")
    gw2c_d = nc.dram_tensor("gw2c", [128, H2], F32, kind="ExternalInput")
    cb2_d = nc.dram_tensor("cb2", [128, H2], F32, kind="ExternalInput")
    w3g_d = nc.dram_tensor("w3g", [128, H2], F32, kind="ExternalInput")
    out_d = nc.dram_tensor("out", [BC, S], F32, kind="ExternalOutput")

    with tile.TileContext(nc) as tc, ExitStack() as ctx:
        cpool = ctx.enter_context(tc.tile_pool(name="consts", bufs=1))
        xtp = ctx.enter_context(tc.tile_pool(name="xt", bufs=1))
        w1p = ctx.enter_context(tc.tile_pool(name="w1", bufs=3))
        hrp = ctx.enter_context(tc.tile_pool(name="hr", bufs=2))
        hrtp = ctx.enter_context(tc.tile_pool(name="hrt", bufs=2))
        sqp = ctx.enter_context(tc.tile_pool(name="sq", bufs=1))
        stp = ctx.enter_context(tc.tile_pool(name="stats", bufs=1))
        smp = ctx.enter_context(tc.tile_pool(name="smalls", bufs=16))
        b1p = ctx.enter_context(tc.tile_pool(name="b1p", bufs=2))
        sm64 = ctx.enter_context(tc.tile_pool(name="sm64", bufs=4))
        krp = ctx.enter_context(tc.tile_pool(name="kern", bufs=2))
        cxp = ctx.enter_context(tc.tile_pool(name="convx", bufs=2))
        ctp = ctx.enter_context(tc.tile_pool(name="convt", bufs=1))
        psp = ctx.enter_context(tc.tile_pool(name="ps", bufs=8, space="PSUM"))

        # ---- constants ----
        ident = cpool.tile([128, 128], F32)
        make_identity(nc, ident[:])
        w2g = cpool.tile([128, JB, H2], F32)
        nc.sync.dma_start(
            w2g[:], w2g_d.ap().rearrange("(j p) n -> p j n", p=128)
        )
        gw2c = cpool.tile([128, H2], F32)
        nc.sync.dma_start(gw2c[:], gw2c_d[:])
        cb2 = cpool.tile([128, H2], F32)
        nc.sync.dma_start(cb2[:], cb2_d[:])
        w3g = cpool.tile([128, H2], F32)
        nc.sync.dma_start(w3g[:], w3g_d[:])
        eps_t = cpool.tile([128, 1], F32)
        nc.vector.memset(eps_t[:], LN_EPS)
        salt_t = cpool.tile([128, 1], F32)
        nc.vector.memset(salt_t[:], 1.0 + SALT)
        dv_t = cpool.tile([128, 1], F32)
        nc.vector.memset(dv_t[:], D_val)

        # ---- resident x^T (one tile per 128-row s-block) ----
        xt_tiles = []
        for so in range(ST):
            t = xtp.tile([128, BC], F32R, tag=f"xt{so}")
            nc.sync.dma_start(t[:], xt_d[so * 128 : (so + 1) * 128, :])
            xt_tiles.append(t)

        # ---- persistent per-batch-tile stats / h2 accumulators ----
        sums = stp.tile([128, NB * HC], F32)
        sqs = stp.tile([128, NB * HC], F32)
        h2sb = stp.tile([128, NB * H2], F32)

        # ---- main pipeline over h1 chunks ----
        for c in range(HC):
            ps_mm1 = [psp.tile([128, 512], F32, tag="ps", name=f"psmm1_{c}_{i}") for i in range(NB)]
            for sg in range(SG):
                w1g = w1p.tile([128, 4, 512], F32R)
                nc.sync.dma_start(
                    w1g[:],
                    w1_d[sg * 512 : (sg + 1) * 512, c * 512 : (c + 1) * 512]
                    .rearrange("(t p) n -> p t n", p=128),
                )
                for t in range(4):
                    so = sg * 4 + t
                    for b in range(NB):
                        nc.tensor.matmul(
                            ps_mm1[b][:],
                            xt_tiles[so][:, b * 128 : (b + 1) * 128],
                            w1g[:, t : t + 1, :],
                            start=(so == 0),
                            stop=(so == ST - 1),
                        )
            b1c = b1p.tile([128, 512], F32, tag="b1c")
            nc.sync.dma_start(b1c[:], b1_d[:, c * 512 : (c + 1) * 512])
            for b in range(NB):
                k = b * HC + c
                hpre = hrp.tile([128, 512], F32, tag="hpre")
                nc.vector.tensor_tensor(
                    hpre[:], ps_mm1[b][:], b1c[:], ALU.add
                )
                hr = hrp.tile([128, 512], F32, tag="hr")
                nc.scalar.activation(
                    hr[:], hpre[:], AF.Relu,
                    accum_out=sums[:, k : k + 1],
                )
                nc.scalar.activation(
                    hpre[:], hr[:], AF.Square,
                    accum_out=sqs[:, k : k + 1],
                )
                if STAGE < 2:
                    continue
                for j in range(4):
                    nc.tensor.transpose(
                        ps_mm1[b][:, j * 128 : (j + 1) * 128],
                        hr[:, j * 128 : (j + 1) * 128],
                        ident[:],
                    )
                hrt = hrtp.tile([128, 512], F32)
                nc.vector.tensor_copy(hrt[:], ps_mm1[b][:])
                for j in range(4):
                    jj = c * 4 + j
                    nc.tensor.matmul(
                        ps_mm1[b][:, 0:H2],
                        hrt[:, j * 128 : (j + 1) * 128],
                        w2g[:, jj : jj + 1, :],
                        start=(j == 0),
                        stop=(j == 3),
                    )
                dst = h2sb[:, b * H2 : (b + 1) * H2]
                if c == 0:
                    nc.vector.tensor_copy(dst, ps_mm1[b][:, 0:H2])
                else:
                    nc.vector.tensor_tensor(dst, dst, ps_mm1[b][:, 0:H2], ALU.add)

        # ---- per-batch-tile tail: LN folds, sigma, kernel, conv ----
        for b in range(NB):
            if STAGE < 3:
                break
            # LN1 stats
            ssum = smp.tile([128, 1], F32, tag="sm")
            nc.vector.tensor_reduce(
                ssum[:], sums[:, b * HC : (b + 1) * HC], axis=AX.X, op=ALU.add
            )
            m = smp.tile([128, 1], F32, tag="sm")
            nc.vector.tensor_scalar_mul(m[:], ssum[:], inv_h1)
            qsum = smp.tile([128, 1], F32, tag="sm")
            nc.vector.tensor_reduce(
                qsum[:], sqs[:, b * HC : (b + 1) * HC], axis=AX.X, op=ALU.add
            )
            msq = smp.tile([128, 1], F32, tag="sm")
            nc.vector.tensor_tensor(msq[:], m[:], m[:], ALU.mult)
            var = smp.tile([128, 1], F32, tag="sm")
            nc.vector.scalar_tensor_tensor(
                var[:], qsum[:], inv_h1, msq[:], ALU.mult, ALU.subtract
            )
            sd = smp.tile([128, 1], F32, tag="sm")
            nc.scalar.activation(sd[:], var[:], AF.Sqrt, bias=eps_t[:])
            rs = smp.tile([128, 1], F32, tag="sm")
            nc.vector.reciprocal(rs[:], sd[:])
            nrs = smp.tile([128, 1], F32, tag="sm")
            nc.vector.tensor_scalar_mul(nrs[:], rs[:], -1.0)
            mnrs = smp.tile([128, 1], F32, tag="sm")
            nc.vector.tensor_tensor(mnrs[:], m[:], nrs[:], ALU.mult)
            if TAIL < 2:
                continue
            # fold LN1 into mm2 output
            u64 = sm64.tile([128, H2], F32, tag="s64")
            nc.vector.tensor_scalar_mul(
                u64[:], h2sb[:, b * H2 : (b + 1) * H2], rs[:]
            )
            w64 = sm64.tile([128, H2], F32, tag="s64")
            nc.vector.scalar_tensor_tensor(
                w64[:], gw2c[:], mnrs[:], u64[:], ALU.mult, ALU.add
            )
            h2p = sm64.tile([128, H2], F32, tag="s64")
            nc.vector.tensor_tensor(h2p[:], w64[:], cb2[:], ALU.add)
            if TAIL < 3:
                continue
            # relu + LN2 stats
            h2r = sm64.tile([128, H2], F32, tag="s64")
            s2t = smp.tile([128, 1], F32, tag="sm")
            nc.scalar.activation(
                h2r[:], h2p[:], AF.Relu, accum_out=s2t[:]
            )
            scr2 = sm64.tile([128, H2], F32, tag="s64")
            q2t = smp.tile([128, 1], F32, tag="sm")
            nc.scalar.activation(
                scr2[:], h2r[:], AF.Square, accum_out=q2t[:]
            )
            m2 = smp.tile([128, 1], F32, tag="sm")
            nc.vector.tensor_scalar_mul(m2[:], s2t[:], inv_h2)
            msq2 = smp.tile([128, 1], F32, tag="sm")
            nc.vector.tensor_tensor(msq2[:], m2[:], m2[:], ALU.mult)
            var2 = smp.tile([128, 1], F32, tag="sm")
            nc.vector.scalar_tensor_tensor(
                var2[:], q2t[:], inv_h2, msq2[:], ALU.mult, ALU.subtract
            )
            sd2 = smp.tile([128, 1], F32, tag="sm")
            nc.scalar.activation(sd2[:], var2[:], AF.Sqrt, bias=eps_t[:])
            rs2 = smp.tile([128, 1], F32, tag="sm")
            nc.vector.reciprocal(rs2[:], sd2[:])
            if TAIL < 4:
                continue
            # mm3 (LN2 folded): logit = rs2*lraw - m2*rs2*C + D
            scr3 = sm64.tile([128, H2], F32, tag="s64")
            nc.vector.tensor_tensor(scr3[:], h2r[:], w3g[:], ALU.mult)
            lraw = smp.tile([128, 1], F32, tag="sm")
            nc.vector.tensor_reduce(lraw[:], scr3[:], axis=AX.X, op=ALU.add)
            s1 = smp.tile([128, 1], F32, tag="sm")
            nc.vector.tensor_tensor(s1[:], lraw[:], rs2[:], ALU.mult)
            s2m = smp.tile([128, 1], F32, tag="sm")
            nc.vector.tensor_tensor(s2m[:], m2[:], rs2[:], ALU.mult)
            lp = smp.tile([128, 1], F32, tag="sm")
            nc.vector.scalar_tensor_tensor(
                lp[:], s2m[:], -C_val, s1[:], ALU.mult, ALU.add
            )
            sig = smp.tile([128, 1], F32, tag="sm")
            nc.scalar.activation(sig[:], lp[:], AF.Sigmoid, bias=dv_t[:])
            if TAIL < 5:
                continue
            # gaussian taps: e1 = exp(-0.5/s^2), e4 = exp(-2/s^2)
            s2sig = smp.tile([128, 1], F32, tag="sm")
            nc.vector.tensor_tensor(s2sig[:], sig[:], sig[:], ALU.mult)
            rinv = smp.tile([128, 1], F32, tag="sm")
            nc.vector.reciprocal(rinv[:], s2sig[:])
            e1 = krp.tile([128, 1], F32, tag="e1")
            nc.scalar.activation(e1[:], rinv[:], AF.Exp, scale=-0.5)
            e4 = krp.tile([128, 1], F32, tag="e4")
            nc.scalar.activation(e4[:], rinv[:], AF.Exp, scale=-2.0)
            esum = smp.tile([128, 1], F32, tag="sm")
            nc.vector.tensor_tensor(esum[:], e1[:], e4[:], ALU.add)
            den = smp.tile([128, 1], F32, tag="sm")
            nc.vector.tensor_scalar(den[:], esum[:], 2.0, 1.0, ALU.mult, ALU.add)
            kc = krp.tile([128, 1], F32, tag="kc")
            nc.vector.reciprocal(kc[:], den[:])
            # conv: out = kc*x + e1*(kc*x)<<>>1 + e4*(kc*x)<<>>2
            for q in range(NQ if STAGE >= 4 else 0):
                xs = cxp.tile([128, CQ + 4], F32, tag="xs")
                nc.sync.dma_start(
                    xs[:],
                    xp_d[b * 128 : (b + 1) * 128, q * CQ : q * CQ + CQ + 4],
                )
                nc.scalar.activation(xs[:], xs[:], AF.Copy, scale=kc[:])
                t1 = ctp.tile([128, CQ], F32, tag="t1")
                nc.vector.tensor_tensor(
                    t1[:], xs[:, 1 : CQ + 1], xs[:, 3 : CQ + 3], ALU.add
                )
                t2 = ctp.tile([128, CQ], F32, tag="t2")
                nc.vector.tensor_tensor(
                    t2[:], xs[:, 0:CQ], xs[:, 4 : CQ + 4], ALU.add
                )
                nc.vector.scalar_tensor_tensor(
                    t1[:], t1[:], e1[:], xs[:, 2 : CQ + 2], ALU.mult, ALU.add
                )
                nc.vector.scalar_tensor_tensor(
                    xs[:, 2 : CQ + 2], t2[:], e4[:], t1[:], ALU.mult, ALU.add
                )
                nc.sync.dma_start(
                    out_d[b * 128 : (b + 1) * 128, q * CQ : (q + 1) * CQ],
                    xs[:, 2 : CQ + 2],
                )

    nc.compile()
    return nc


def prep_inputs(x, W1, b1, g1, be1, W2, b2, g2, be2, W3, b3, n_cores=N_CORES):
    """Host-side sharding + derived-constant computation."""
    B, S = x.shape
    H1 = W1.shape[1]
    BC = B // n_cores
    f = np.float32

    W2g = (W2 * g1[:, None]).astype(f)  # [H1, H2]
    gw2c = np.tile((g1 @ W2).astype(f)[None, :], (128, 1)).astype(f)
    cb2 = np.tile((be1 @ W2 + b2).astype(f)[None, :], (128, 1)).astype(f)
    w3g = np.tile((g2 * W3[:, 0]).astype(f)[None, :], (128, 1)).astype(f)
    C_val = float(np.sum(g2 * W3[:, 0], dtype=np.float64))
    D_val = float(np.dot(be2, W3[:, 0]) + b3[0])

    in_maps = []
    for c in range(n_cores):
        xs = x[c * BC : (c + 1) * BC]
        xt = np.ascontiguousarray(xs.T)
        xp = np.zeros((BC, S + 4), f)
        xp[:, 2 : S + 2] = xs
        in_maps.append(
            {
                "xt": xt,
                "xp": xp,
                "w1": np.ascontiguousarray(W1.astype(f)),
                "b1b": np.ascontiguousarray(
                    np.broadcast_to(b1.astype(f)[None, :], (128, len(b1)))
                ),
                "w2g": W2g,
                "gw2c": gw2c,
                "cb2": cb2,
                "w3g": w3g,
            }
        )
    return in_maps, C_val, D_val, BC, S, H1


def _kernel_impl(x, W1, b1, g1, be1, W2, b2, g2, be2, W3, b3):
    in_maps, C_val, D_val, BC, S, H1 = prep_inputs(
        x, W1, b1, g1, be1, W2, b2, g2, be2, W3, b3
    )
    nc = build_program(BC, S, H1, C_val, D_val)
    res = run_bass_kernel_spmd(nc, in_maps, core_ids=list(range(N_CORES)))
    return np.concatenate([r["out"] for r in res.results], axis=0)


def kernel(x, W1, b1, g1, be1, W2, b2, g2, be2, W3, b3):
    """Full-input entry point. On a wedged accelerator worker the jax mesh
    cannot recover in-process, so failed attempts are retried in a fresh
    subprocess (fresh axon client -> recycled device worker)."""
    import os
    import subprocess
    import tempfile
    import time as _time

    args = dict(
        x=x, W1=W1, b1=b1, g1=g1, be1=be1, W2=W2, b2=b2, g2=g2, be2=be2,
        W3=W3, b3=b3,
    )
    try:
        return _kernel_impl(**args)
    except Exception:
        pass
    last_err = None
    for _ in range(3):
        _time.sleep(5)
        tmpd = tempfile.mkdtemp()
        in_npz = os.path.join(tmpd, "in.npz")
        out_npz = os.path.join(tmpd, "out.npz")
        np.savez(in_npz, **args)
        try:
            subprocess.run(
                [sys.executable, os.path.abspath(__file__), in_npz, out_npz],
                check=True, timeout=1800,
            )
            return np.load(out_npz)["out"]
        except Exception as e:
            last_err = e
    raise last_err


def _worker_main(in_npz, out_npz):
    args = dict(np.load(in_npz))
    out = _kernel_impl(**args)
    np.savez(out_npz, out=out)


if __name__ == "__main__":
    _worker_main(sys.argv[1], sys.argv[2])
